# revision 45
# baseline (speedup 1.0000x reference)
"""Trainium2 Bass kernel for nn_AttentionLayer (b=4, l=s=2048, D=64, H=8, hd=8).

Sharding: 8 cores = 4 batches x 2 head-quads (4 heads each). Every core runs
the identical SPMD program over its batch's full causal triangle for its 4
heads; the host sums the two per-batch output-projection partials (standard
tensor-parallel reduction over heads).

Per-core dataflow (fp32 accumulate; f32r projections, bf16 q/k/v/attn):
  qT/kT = W_pad @ x_augT   (f32r matmuls; heads padded to 32-partition row groups)
  scoresT[s,l] per head via bf16 row-tiled matmuls (K=8, tile_position=(32r,0))
  exp on ScalarE (PSUM->SBUF bf16); on 12 spans a custom 2-pass vector-engine
  op (degree-4 Horner on x/8, then 3 squarings, 6.4e-5 rel err) takes half the
  span so ACT and DVE pipeline; causal diagonal masked by a tri multiply on DVE
  AV^T + denominator via bf16 col-tiled matmuls with [V|1] stationary operand,
  three spans software-pipelined behind the exp
  normalize with reciprocal_approx_fast + PE broadcast matmul
  y^T = Wo_pad @ av_norm + bo/2 (f32r), accumulated in PSUM, DMA'd out
"""

import os
import sys

for _p in ("/opt/trn_rl_repo", "/root/.axon_site/_ro/trn_rl_repo"):
    if os.path.isdir(_p) and _p not in sys.path:
        sys.path.append(_p)

import numpy as np

H = 8
D = 64
HD = 8
B = 4
L = 2048
SCALE = 1.0 / np.sqrt(np.float32(HD))

NT = L // 128   # 16 s-tiles of 128
NG = 4          # groups of 512 query columns
F32 = None      # set after imports

_CACHE = {}
LAST_EXEC_NS = None

# minimax exp(x) ~= (1 + x(B1 + x(B2 + x(B3 + x B4))))**8 on [-3.4, 3.4],
# max rel err 6.4e-5 (pass1 = degree-4 Horner, pass2 = 3 squarings)
EXP_B = (0.12499097, 7.81358073e-03, 3.29033668e-04, 1.01144194e-05)
_EXP_OPS = None


def _register_exp_ops():
    global _EXP_OPS
    if _EXP_OPS is not None:
        return _EXP_OPS
    import concourse.dve_ops as dops
    from concourse.dve_spec import (
        Spec, Src0, C0, C1, C2, C3, One, sq, lower, _spill_c3_to_src1,
        _has_src1 as has_src1,
    )
    from concourse.dve_uop import DveOpSpec

    def make(name, spec):
        for o in dops.OPS:
            if o.name == name:
                return o
        op = dops.DveOp(name, spec, subdim=False, uops_sha={})
        dops.OPS.append(op)
        dops._SUB_OPCODE_FOR_NAME[name] = dops._CUSTOM_DVE_ROW_BASE + len(dops.OPS) - 1
        assert max(dops._SUB_OPCODE_FOR_NAME.values()) < 0x20
        for ver in ("v3", "v4"):
            r = DveOpSpec(name=name, opcode=dops.get_dve_sub_opcode(name),
                          uops=lower(spec, ver=ver), rd1_en=has_src1(spec))
            op.uops_sha[ver] = r.sha(ver)
        return op

    b1, b2, b3, b4 = (float(v) for v in EXP_B)
    body1 = One + Src0 * (C0 + Src0 * (C1 + Src0 * (C2 + Src0 * C3)))
    p4 = make("EXP_P4_ANT", Spec(
        body=_spill_c3_to_src1(body1),
        reference=lambda in0, in1, s0, s1, imm2:
            1.0 + in0 * (s0 + in0 * (s1 + in0 * (imm2 + in0 * in1))),
    ))
    sq3 = make("SQ3_ANT", Spec(
        body=sq(sq(sq(Src0))),
        reference=lambda in0: ((in0 * in0) ** 2) ** 2,
    ))
    _EXP_OPS = (p4, sq3)
    return _EXP_OPS


# (group, s-tile) spans whose exp runs on the vector engine instead of ACT
DVE_SPANS = {(1, 1), (2, 0), (2, 2), (2, 4), (2, 6), (3, 0), (3, 2), (3, 4),
             (3, 6), (3, 8), (3, 10), (3, 11)}



def _build(causal: bool):
    import concourse.bacc as bacc
    import concourse.tile as tile
    import concourse.mybir as mybir

    f32 = mybir.dt.float32
    f32r = mybir.dt.float32r
    bf16 = mybir.dt.bfloat16
    AF = mybir.ActivationFunctionType

    exp_p4, exp_sq3 = _register_exp_ops()
    nc = bacc.Bacc("TRN2", target_bir_lowering=False, debug=False, num_devices=8)

    xq_d = nc.dram_tensor("xq", [65, L], f32r, kind="ExternalInput").ap()
    xk_d = nc.dram_tensor("xk", [65, L], f32r, kind="ExternalInput").ap()
    xv_d = nc.dram_tensor("xv", [65, L], f32r, kind="ExternalInput").ap()
    wq_d = nc.dram_tensor("wq", [65, 128], f32r, kind="ExternalInput").ap()
    wk_d = nc.dram_tensor("wk", [65, 128], f32r, kind="ExternalInput").ap()
    wv_d = nc.dram_tensor("wv", [65, 36], f32r, kind="ExternalInput").ap()
    wo_d = nc.dram_tensor("wo", [128, 64], f32r, kind="ExternalInput").ap()
    bo_d = nc.dram_tensor("bo2", [1, 64], f32r, kind="ExternalInput").ap()
    tri_d = nc.dram_tensor("tri", [128, 512], bf16, kind="ExternalInput").ap()
    sel_d = nc.dram_tensor("sel", [128, 128], f32r, kind="ExternalInput").ap()
    ones_d = nc.dram_tensor("ones", [1, 512], f32r, kind="ExternalInput").ap()
    y_d = nc.dram_tensor("y", [64, L], f32, kind="ExternalOutput").ap()

    from contextlib import ExitStack

    with tile.TileContext(nc) as tc, ExitStack() as es:
        singles = es.enter_context(tc.tile_pool(name="singles", bufs=1))
        persist = es.enter_context(tc.tile_pool(name="persist", bufs=1))
        sc_ps = es.enter_context(tc.tile_pool(name="sc_ps", bufs=3, space="PSUM"))
        av_ps_pool = es.enter_context(tc.tile_pool(name="av_ps", bufs=1, space="PSUM"))
        misc_ps = es.enter_context(tc.tile_pool(name="misc_ps", bufs=1, space="PSUM"))
        attn_pool = es.enter_context(tc.tile_pool(name="attn", bufs=8))
        small = es.enter_context(tc.tile_pool(name="small", bufs=2))

        # ---- load inputs ----
        xq = singles.tile([65, L], f32r, tag="xq")
        xk = singles.tile([65, L], f32r, tag="xk")
        xv = singles.tile([65, L], f32r, tag="xv")
        wq = singles.tile([65, 128], f32r, tag="wq")
        wk = singles.tile([65, 128], f32r, tag="wk")
        wv = singles.tile([65, 36], f32r, tag="wv")
        wo = singles.tile([128, 64], f32r, tag="wo")
        bo2 = singles.tile([1, 64], f32r, tag="bo2")
        tri = singles.tile([128, 512], bf16, tag="tri")
        sel = singles.tile([128, 128], f32r, tag="sel")
        ones_row = singles.tile([1, 512], f32r, tag="ones_row")

        # weights on the scalar queue; ones first so the dummy exp (which
        # preloads the ACT table) can run during the rest of the DMA
        nc.scalar.dma_start(out=wk[:], in_=wk_d[:])
        nc.scalar.dma_start(out=ones_row[:], in_=ones_d[:])
        nc.scalar.dma_start(out=wq[:], in_=wq_d[:])
        nc.scalar.dma_start(out=wv[:], in_=wv_d[:])
        for t_, d_ in ((wo, wo_d), (bo2, bo_d), (sel, sel_d)):
            nc.gpsimd.dma_start(out=t_[:], in_=d_[:])
        nc.gpsimd.dma_start(out=tri[:], in_=tri_d[:])
        warm = singles.tile([1, 512], f32, tag="warm")
        nc.scalar.activation(out=warm[:], in_=ones_row[:].bitcast(f32), func=AF.Exp)
        c3t = singles.tile([128, 1], f32, tag="c3t")
        nc.vector.memset(c3t[:], float(EXP_B[3]))

        # ---- chunked input DMA + projections, interleaved ----
        qT = persist.tile([128, L], bf16, tag="qT")
        kT = persist.tile([128, L], bf16, tag="kT")
        v_aug = persist.tile([128, 16 * 36], bf16, tag="v_aug")

        for j in range(4):
            sl = slice(512 * j, 512 * (j + 1))
            nc.sync.dma_start(out=xk[:, sl], in_=xk_d[:, sl])
            nc.sync.dma_start(out=xq[:, sl], in_=xq_d[:, sl])
            nc.scalar.dma_start(out=xv[:, sl], in_=xv_d[:, sl])

        def proj(j):
            # chunk-j projections, emitted just before the attention group
            # that first needs them so PSUM slot allocation can't stall
            # ready attention work behind DMA-blocked projections
            sl = slice(512 * j, 512 * (j + 1))
            pj = sc_ps.tile([128, 1024], f32, tag="sc")
            nc.tensor.matmul(out=pj[:, :512], lhsT=wk[:], rhs=xk[:, sl],
                             start=True, stop=True)
            nc.vector.tensor_copy(kT[:, sl], pj[:, :512])
            pj = sc_ps.tile([128, 1024], f32, tag="sc")
            nc.tensor.matmul(out=pj[:, :512], lhsT=wq[:], rhs=xq[:, sl],
                             start=True, stop=True)
            nc.vector.tensor_copy(qT[:, sl], pj[:, :512])
            pj = sc_ps.tile([128, 1024], f32, tag="sc")
            for tt in range(4):
                t = 4 * j + tt
                nc.tensor.matmul(out=pj[:, 256 * tt:256 * tt + 36],
                                 lhsT=xv[:, 128 * t:128 * (t + 1)], rhs=wv[:],
                                 start=True, stop=True)
            nc.vector.tensor_copy(
                v_aug.rearrange("p (c n) -> p c n", n=36)[:, 4 * j:4 * j + 4, :],
                pj.rearrange("p (c n) -> p c n", n=256)[:, :, :36])

        # ---- attention (chunk-j projections interleaved per group) ----
        for g in range(NG):
            proj(g)
            n_s = 4 * g + 4 if causal else NT
            # 1.0 (not 0) so junk rows stay finite through reciprocal below
            av = av_ps_pool.tile([128, 512], f32, tag="av")
            nc.vector.memset(av[:], 1.0)
            pending_av = []

            def emit_av(t, col_off, attnT):
                for c in range(4):
                    nc.tensor.matmul(
                        out=av[32 * c:32 * c + 9, col_off:512],
                        lhsT=v_aug[:, 36 * t + 9 * c:36 * t + 9 * c + 9],
                        rhs=attnT[:, 512 * c + col_off:512 * (c + 1)],
                        start=(t == 0), stop=(t == n_s - 1),
                        tile_position=(0, 32 * c))

            for t in range(n_s):
                col_off = max(0, (t - 4 * g) * 128) if causal else 0
                npr = 512 - col_off
                scA = sc_ps.tile([128, 1024], f32, tag="sc")
                scB = sc_ps.tile([128, 1024], f32, tag="sc")
                for r in range(4):
                    sc = scA if r < 2 else scB
                    c0 = 512 * (r % 2) + col_off
                    nc.tensor.matmul(
                        out=sc[:, c0:c0 + npr],
                        lhsT=kT[32 * r:32 * r + 8, 128 * t:128 * (t + 1)],
                        rhs=qT[32 * r:32 * r + 8, 512 * g + col_off:512 * (g + 1)],
                        start=True, stop=True, tile_position=(32 * r, 0))
                attnT = attn_pool.tile([128, 2048], bf16, tag="attnT")
                a4 = attnT.rearrange("p (h n) -> p h n", h=4)
                # on DVE-assigned (non-diag) spans the custom 2-pass poly exp
                # takes heads 0-1 while ACT takes heads 2-3; otherwise ACT all
                if causal and (g, t) in DVE_SPANS and t < 4 * g:
                    scr = attn_pool.tile([128, 1024], f32, tag="scr")
                    nc.vector._custom_dve(
                        exp_p4, out=scr[:], in0=scA[:, :1024],
                        s0=float(EXP_B[0]), s1=float(EXP_B[1]),
                        imm2=float(EXP_B[2]), in1=c3t[:])
                    nc.vector._custom_dve(
                        exp_sq3, out=attnT[:, 0:1024], in0=scr[:])
                    s2 = scB.rearrange("p (h n) -> p h n", h=2)
                    nc.scalar.activation(
                        out=a4[:, 2:4, col_off:512],
                        in_=s2[:, :, col_off:512],
                        func=AF.Exp)
                else:
                    for p_ in range(2):
                        sc = scA if p_ == 0 else scB
                        s2 = sc.rearrange("p (h n) -> p h n", h=2)
                        nc.scalar.activation(
                            out=a4[:, 2 * p_:2 * p_ + 2, col_off:512],
                            in_=s2[:, :, col_off:512],
                            func=AF.Exp)
                if causal and t >= 4 * g:
                    t4 = tri.rearrange("p (h n) -> p h n", h=4)
                    nc.vector.tensor_mul(
                        a4[:, :, col_off:col_off + 128],
                        a4[:, :, col_off:col_off + 128],
                        t4[:, :, :])
                pending_av.append((t, col_off, attnT))
                if len(pending_av) > 4:
                    emit_av(*pending_av.pop(0))
            for args in pending_av:
                emit_av(*args)

            # ---- group epilogue: normalize + output projection ----
            av_sb = small.tile([128, 512], f32, tag="av_sb")
            nc.vector.tensor_copy(av_sb[:], av[:])
            rcp = small.tile([128, 512], f32, tag="rcp")
            nc.vector.reciprocal_approx_fast(out=rcp[:], in_=av_sb[:])
            bc = misc_ps.tile([128, 512], f32, tag="misc")
            nc.tensor.matmul(out=bc[:], lhsT=sel[:].bitcast(f32), rhs=rcp[:],
                             start=True, stop=True)
            avn = small.tile([128, 512], f32r, tag="avn")
            nc.vector.tensor_mul(avn[:], av_sb[:], bc[:])
            yp = misc_ps.tile([64, 512], f32, tag="misc")
            nc.tensor.matmul(out=yp[:], lhsT=wo[:], rhs=avn[:], start=True, stop=False)
            nc.tensor.matmul(out=yp[:], lhsT=bo2[:], rhs=ones_row[:], start=False, stop=True)
            y_sb = small.tile([64, 512], f32, tag="y_sb")
            nc.vector.tensor_copy(y_sb[:], yp[:])
            nc.sync.dma_start(out=y_d[:, 512 * g:512 * (g + 1)], in_=y_sb[:])

    nc.compile()
    return nc


def _prep_inputs(queries, keys, values, Wq, bq, Wk, bk, Wv, bv, Wo, bo):
    """Build the 8 per-core input maps (host-side layout/sharding only)."""
    ones = np.ones((1, L), np.float32)

    def aug_t(x_b):  # [L, 64] -> [65, L]
        return np.ascontiguousarray(np.vstack([x_b.T, ones]).astype(np.float32))

    # padded projection weights per quad: col 32r+d <- head (4Q+r) dim d
    def w_pad(W, b, quad, scale=1.0):
        out = np.zeros((65, 128), np.float32)
        for r in range(4):
            ch = 8 * (4 * quad + r)
            out[:64, 32 * r:32 * r + 8] = W[ch:ch + 8, :].T * scale
            out[64, 32 * r:32 * r + 8] = b[ch:ch + 8] * scale
        return out

    def wv_aug(quad):  # [65, 36]: col 9c+e <- head (4Q+c) dim e; col 9c+8 = e64
        out = np.zeros((65, 36), np.float32)
        for c in range(4):
            ch = 8 * (4 * quad + c)
            out[:64, 9 * c:9 * c + 8] = Wv[ch:ch + 8, :].T
            out[64, 9 * c:9 * c + 8] = bv[ch:ch + 8]
            out[64, 9 * c + 8] = 1.0
        return out

    def wo_pad(quad):  # [128, 64]: row 32c+d -> Wo[:, 8(4Q+c)+d]
        out = np.zeros((128, 64), np.float32)
        for c in range(4):
            ch = 8 * (4 * quad + c)
            out[32 * c:32 * c + 8, :] = Wo[:, ch:ch + 8].T
        return out

    import ml_dtypes
    tri = (np.arange(128)[:, None] <= np.arange(128)[None, :]).astype(np.float32)
    tri4 = np.ascontiguousarray(np.tile(tri, (1, 4)).astype(ml_dtypes.bfloat16))
    sel = np.zeros((128, 128), np.float32)
    for c in range(4):
        sel[32 * c + 8, 32 * c:32 * c + 9] = 1.0
    bo2 = (bo.astype(np.float32) / 2.0).reshape(1, 64)

    w_cache = {}
    for q in range(2):
        w_cache[q] = dict(
            wq=w_pad(Wq, bq, q, scale=float(SCALE)),
            wk=w_pad(Wk, bk, q),
            wv=wv_aug(q),
            wo=wo_pad(q),
        )

    in_maps = []
    for c in range(8):
        b, q = c // 2, c % 2
        in_maps.append(dict(
            xq=aug_t(np.asarray(queries[b])),
            xk=aug_t(np.asarray(keys[b])),
            xv=aug_t(np.asarray(values[b])),
            wq=w_cache[q]["wq"], wk=w_cache[q]["wk"],
            wv=w_cache[q]["wv"], wo=w_cache[q]["wo"],
            bo2=bo2, tri=tri4, sel=sel, ones=np.ones((1, 512), np.float32),
        ))
    return in_maps


def _install_trace_hook():
    import contextlib
    import ctypes
    import types

    name = "antenv.axon_hooks"
    if name in sys.modules:
        return
    so_path = "/opt/axon/libaxon_pjrt.so"
    if not os.path.exists(so_path):
        return
    lib = ctypes.CDLL(so_path)
    if not hasattr(lib, "axon_start_nrt_profile"):
        return
    lib.axon_start_nrt_profile.argtypes = [ctypes.POINTER(ctypes.c_int64), ctypes.c_size_t]
    lib.axon_start_nrt_profile.restype = ctypes.c_int64
    lib.axon_stop_nrt_profile.argtypes = [ctypes.c_char_p]
    lib.axon_stop_nrt_profile.restype = ctypes.c_int64

    @contextlib.contextmanager
    def _hook(output_dir, device_ids):
        import jax
        jax.devices()
        if device_ids:
            ids = (ctypes.c_int64 * len(device_ids))(*device_ids)
            rc = lib.axon_start_nrt_profile(ids, len(device_ids))
        else:
            rc = lib.axon_start_nrt_profile(None, 0)
        if rc != 0:
            raise RuntimeError(f"axon_start_nrt_profile rc={rc}")
        try:
            yield
        finally:
            n = lib.axon_stop_nrt_profile(str(output_dir).encode())
            print(f"profile: {n} file(s) in {output_dir}", file=sys.stderr)

    mod = types.ModuleType(name)
    mod._hook = _hook
    mod.set_axon_ntff_profile_hook = lambda h: setattr(mod, "_hook", h)
    mod.get_axon_ntff_profile_hook = lambda: mod._hook
    sys.modules[name] = mod


def kernel(queries, keys, values, attention_mask, Wq, bq, Wk, bk, Wv, bv, Wo, bo):
    global LAST_EXEC_NS
    from concourse.bass_utils import run_bass_kernel_spmd

    causal = bool(int(np.asarray(attention_mask)))
    if causal not in _CACHE:
        _CACHE[causal] = _build(causal)
    nc = _CACHE[causal]

    in_maps = _prep_inputs(queries, keys, values, Wq, bq, Wk, bk, Wv, bv, Wo, bo)

    trace = os.environ.get("KERNEL_TRACE", "") == "1"
    kwargs = {}
    if trace:
        _install_trace_hook()
        kwargs = dict(trace=True, tmpdir=os.environ.get("KERNEL_TRACE_DIR") or None)
    res = run_bass_kernel_spmd(nc, in_maps, core_ids=list(range(8)), **kwargs)
    LAST_EXEC_NS = res.exec_time_ns

    out = np.empty((B, L, D), np.float32)
    for b in range(B):
        out[b] = (res.results[2 * b]["y"] + res.results[2 * b + 1]["y"]).T
    return out


# revision 46
# speedup vs baseline: 1.0105x; 1.0105x over previous
"""Trainium2 Bass kernel for nn_AttentionLayer (b=4, l=s=2048, D=64, H=8, hd=8).

Sharding: 8 cores = 4 batches x 2 head-quads (4 heads each). Every core runs
the identical SPMD program over its batch's full causal triangle for its 4
heads; the host sums the two per-batch output-projection partials (standard
tensor-parallel reduction over heads).

Per-core dataflow (fp32 accumulate; f32r projections, bf16 q/k/v/attn):
  qT/kT = W_pad @ x_augT   (f32r matmuls; heads padded to 32-partition row groups)
  scoresT[s,l] per head via bf16 row-tiled matmuls (K=8, tile_position=(32r,0))
  exp on ScalarE (PSUM->SBUF bf16); on 12 spans a custom 2-pass vector-engine
  op (degree-4 Horner on x/8, then 3 squarings, 6.4e-5 rel err) takes half the
  span so ACT and DVE pipeline; causal diagonal masked by a tri multiply on DVE
  AV^T + denominator via bf16 col-tiled matmuls with [V|1] stationary operand,
  three spans software-pipelined behind the exp
  normalize with reciprocal_approx_fast + PE broadcast matmul
  y^T = Wo_pad @ av_norm + bo/2 (f32r), accumulated in PSUM, DMA'd out
"""

import os
import sys

for _p in ("/opt/trn_rl_repo", "/root/.axon_site/_ro/trn_rl_repo"):
    if os.path.isdir(_p) and _p not in sys.path:
        sys.path.append(_p)

import numpy as np

H = 8
D = 64
HD = 8
B = 4
L = 2048
SCALE = 1.0 / np.sqrt(np.float32(HD))

NT = L // 128   # 16 s-tiles of 128
NG = 4          # groups of 512 query columns
F32 = None      # set after imports

_CACHE = {}
LAST_EXEC_NS = None

# minimax exp(x) ~= (1 + x(B1 + x(B2 + x(B3 + x B4))))**8 on [-3.4, 3.4],
# max rel err 6.4e-5 (pass1 = degree-4 Horner, pass2 = 3 squarings)
EXP_B = (0.12499097, 7.81358073e-03, 3.29033668e-04, 1.01144194e-05)
_EXP_OPS = None


def _register_exp_ops():
    global _EXP_OPS
    if _EXP_OPS is not None:
        return _EXP_OPS
    import concourse.dve_ops as dops
    from concourse.dve_spec import (
        Spec, Src0, C0, C1, C2, C3, One, sq, lower, _spill_c3_to_src1,
        _has_src1 as has_src1,
    )
    from concourse.dve_uop import DveOpSpec

    def make(name, spec):
        for o in dops.OPS:
            if o.name == name:
                return o
        op = dops.DveOp(name, spec, subdim=False, uops_sha={})
        dops.OPS.append(op)
        dops._SUB_OPCODE_FOR_NAME[name] = dops._CUSTOM_DVE_ROW_BASE + len(dops.OPS) - 1
        assert max(dops._SUB_OPCODE_FOR_NAME.values()) < 0x20
        for ver in ("v3", "v4"):
            r = DveOpSpec(name=name, opcode=dops.get_dve_sub_opcode(name),
                          uops=lower(spec, ver=ver), rd1_en=has_src1(spec))
            op.uops_sha[ver] = r.sha(ver)
        return op

    b1, b2, b3, b4 = (float(v) for v in EXP_B)
    body1 = One + Src0 * (C0 + Src0 * (C1 + Src0 * (C2 + Src0 * C3)))
    p4 = make("EXP_P4_ANT", Spec(
        body=_spill_c3_to_src1(body1),
        reference=lambda in0, in1, s0, s1, imm2:
            1.0 + in0 * (s0 + in0 * (s1 + in0 * (imm2 + in0 * in1))),
    ))
    sq3 = make("SQ3_ANT", Spec(
        body=sq(sq(sq(Src0))),
        reference=lambda in0: ((in0 * in0) ** 2) ** 2,
    ))
    _EXP_OPS = (p4, sq3)
    return _EXP_OPS


# (group, s-tile) spans whose exp runs on the vector engine instead of ACT
DVE_SPANS = {(1, 1), (2, 0), (2, 2), (2, 4), (2, 6), (3, 0), (3, 2), (3, 4),
             (3, 6), (3, 8), (3, 10), (3, 11)}



def _build(causal: bool):
    import concourse.bacc as bacc
    import concourse.tile as tile
    import concourse.mybir as mybir

    f32 = mybir.dt.float32
    f32r = mybir.dt.float32r
    bf16 = mybir.dt.bfloat16
    AF = mybir.ActivationFunctionType

    exp_p4, exp_sq3 = _register_exp_ops()
    nc = bacc.Bacc("TRN2", target_bir_lowering=False, debug=False, num_devices=8)

    xq_d = nc.dram_tensor("xq", [65, L], f32r, kind="ExternalInput").ap()
    xk_d = nc.dram_tensor("xk", [65, L], f32r, kind="ExternalInput").ap()
    xv_d = nc.dram_tensor("xv", [65, L], f32r, kind="ExternalInput").ap()
    wq_d = nc.dram_tensor("wq", [65, 128], f32r, kind="ExternalInput").ap()
    wk_d = nc.dram_tensor("wk", [65, 128], f32r, kind="ExternalInput").ap()
    wv_d = nc.dram_tensor("wv", [65, 36], f32r, kind="ExternalInput").ap()
    wo_d = nc.dram_tensor("wo", [128, 64], f32r, kind="ExternalInput").ap()
    bo_d = nc.dram_tensor("bo2", [1, 64], f32r, kind="ExternalInput").ap()
    tri_d = nc.dram_tensor("tri", [128, 512], bf16, kind="ExternalInput").ap()
    sel_d = nc.dram_tensor("sel", [128, 128], f32r, kind="ExternalInput").ap()
    ones_d = nc.dram_tensor("ones", [1, 512], f32r, kind="ExternalInput").ap()
    y_d = nc.dram_tensor("y", [64, L], f32, kind="ExternalOutput").ap()

    from contextlib import ExitStack

    with tile.TileContext(nc) as tc, ExitStack() as es:
        singles = es.enter_context(tc.tile_pool(name="singles", bufs=1))
        persist = es.enter_context(tc.tile_pool(name="persist", bufs=1))
        sc_ps = es.enter_context(tc.tile_pool(name="sc_ps", bufs=3, space="PSUM"))
        av_ps_pool = es.enter_context(tc.tile_pool(name="av_ps", bufs=1, space="PSUM"))
        misc_ps = es.enter_context(tc.tile_pool(name="misc_ps", bufs=1, space="PSUM"))
        attn_pool = es.enter_context(tc.tile_pool(name="attn", bufs=6))
        small = es.enter_context(tc.tile_pool(name="small", bufs=2))

        # ---- load inputs ----
        xq = singles.tile([65, L], f32r, tag="xq")
        xk = singles.tile([65, L], f32r, tag="xk")
        xv = singles.tile([65, L], f32r, tag="xv")
        wq = singles.tile([65, 128], f32r, tag="wq")
        wk = singles.tile([65, 128], f32r, tag="wk")
        wv = singles.tile([65, 36], f32r, tag="wv")
        wo = singles.tile([128, 64], f32r, tag="wo")
        bo2 = singles.tile([1, 64], f32r, tag="bo2")
        tri = singles.tile([128, 512], bf16, tag="tri")
        sel = singles.tile([128, 128], f32r, tag="sel")
        ones_row = singles.tile([1, 512], f32r, tag="ones_row")

        # weights on the scalar queue; ones first so the dummy exp (which
        # preloads the ACT table) can run during the rest of the DMA
        nc.scalar.dma_start(out=wk[:], in_=wk_d[:])
        nc.scalar.dma_start(out=ones_row[:], in_=ones_d[:])
        nc.scalar.dma_start(out=wq[:], in_=wq_d[:])
        nc.scalar.dma_start(out=wv[:], in_=wv_d[:])
        for t_, d_ in ((wo, wo_d), (bo2, bo_d), (sel, sel_d)):
            nc.gpsimd.dma_start(out=t_[:], in_=d_[:])
        nc.gpsimd.dma_start(out=tri[:], in_=tri_d[:])
        warm = singles.tile([1, 512], f32, tag="warm")
        nc.scalar.activation(out=warm[:], in_=ones_row[:].bitcast(f32), func=AF.Exp)
        c3t = singles.tile([128, 1], f32, tag="c3t")
        nc.vector.memset(c3t[:], float(EXP_B[3]))

        # ---- chunked input DMA + projections, interleaved ----
        qT = persist.tile([128, L], bf16, tag="qT")
        kT = persist.tile([128, L], bf16, tag="kT")
        v_aug = persist.tile([128, 16 * 36], bf16, tag="v_aug")

        for j in range(4):
            sl = slice(512 * j, 512 * (j + 1))
            nc.sync.dma_start(out=xk[:, sl], in_=xk_d[:, sl])
            nc.sync.dma_start(out=xq[:, sl], in_=xq_d[:, sl])
            nc.scalar.dma_start(out=xv[:, sl], in_=xv_d[:, sl])

        def proj(j):
            # chunk-j projections, emitted just before the attention group
            # that first needs them so PSUM slot allocation can't stall
            # ready attention work behind DMA-blocked projections
            sl = slice(512 * j, 512 * (j + 1))
            pj = sc_ps.tile([128, 1024], f32, tag="sc")
            nc.tensor.matmul(out=pj[:, :512], lhsT=wk[:], rhs=xk[:, sl],
                             start=True, stop=True)
            nc.vector.tensor_copy(kT[:, sl], pj[:, :512])
            pj = sc_ps.tile([128, 1024], f32, tag="sc")
            nc.tensor.matmul(out=pj[:, :512], lhsT=wq[:], rhs=xq[:, sl],
                             start=True, stop=True)
            nc.vector.tensor_copy(qT[:, sl], pj[:, :512])
            pj = sc_ps.tile([128, 1024], f32, tag="sc")
            for tt in range(4):
                t = 4 * j + tt
                nc.tensor.matmul(out=pj[:, 256 * tt:256 * tt + 36],
                                 lhsT=xv[:, 128 * t:128 * (t + 1)], rhs=wv[:],
                                 start=True, stop=True)
            nc.vector.tensor_copy(
                v_aug.rearrange("p (c n) -> p c n", n=36)[:, 4 * j:4 * j + 4, :],
                pj.rearrange("p (c n) -> p c n", n=256)[:, :, :36])

        # ---- attention (chunk-j projections interleaved per group) ----
        for g in range(NG):
            proj(g)
            n_s = 4 * g + 4 if causal else NT
            # 1.0 (not 0) so junk rows stay finite through reciprocal below
            av = av_ps_pool.tile([128, 512], f32, tag="av")
            nc.vector.memset(av[:], 1.0)
            pending_av = []

            def emit_av(t, col_off, attnT):
                for c in range(4):
                    nc.tensor.matmul(
                        out=av[32 * c:32 * c + 9, col_off:512],
                        lhsT=v_aug[:, 36 * t + 9 * c:36 * t + 9 * c + 9],
                        rhs=attnT[:, 512 * c + col_off:512 * (c + 1)],
                        start=(t == 0), stop=(t == n_s - 1),
                        tile_position=(0, 32 * c))

            for t in range(n_s):
                col_off = max(0, (t - 4 * g) * 128) if causal else 0
                npr = 512 - col_off
                scA = sc_ps.tile([128, 1024], f32, tag="sc")
                scB = sc_ps.tile([128, 1024], f32, tag="sc")
                for r in range(4):
                    sc = scA if r < 2 else scB
                    c0 = 512 * (r % 2) + col_off
                    nc.tensor.matmul(
                        out=sc[:, c0:c0 + npr],
                        lhsT=kT[32 * r:32 * r + 8, 128 * t:128 * (t + 1)],
                        rhs=qT[32 * r:32 * r + 8, 512 * g + col_off:512 * (g + 1)],
                        start=True, stop=True, tile_position=(32 * r, 0))
                attnT = attn_pool.tile([128, 2048], bf16, tag="attnT")
                a4 = attnT.rearrange("p (h n) -> p h n", h=4)
                # on DVE-assigned (non-diag) spans the custom 2-pass poly exp
                # takes heads 0-1 while ACT takes heads 2-3; otherwise ACT all
                if causal and (g, t) in DVE_SPANS and t < 4 * g:
                    scr = attn_pool.tile([128, 1024], f32, tag="scr")
                    nc.vector._custom_dve(
                        exp_p4, out=scr[:], in0=scA[:, :1024],
                        s0=float(EXP_B[0]), s1=float(EXP_B[1]),
                        imm2=float(EXP_B[2]), in1=c3t[:])
                    nc.vector._custom_dve(
                        exp_sq3, out=attnT[:, 0:1024], in0=scr[:])
                    s2 = scB.rearrange("p (h n) -> p h n", h=2)
                    nc.scalar.activation(
                        out=a4[:, 2:4, col_off:512],
                        in_=s2[:, :, col_off:512],
                        func=AF.Exp)
                else:
                    for p_ in range(2):
                        sc = scA if p_ == 0 else scB
                        s2 = sc.rearrange("p (h n) -> p h n", h=2)
                        nc.scalar.activation(
                            out=a4[:, 2 * p_:2 * p_ + 2, col_off:512],
                            in_=s2[:, :, col_off:512],
                            func=AF.Exp)
                if causal and t >= 4 * g:
                    t4 = tri.rearrange("p (h n) -> p h n", h=4)
                    nc.vector.tensor_mul(
                        a4[:, :, col_off:col_off + 128],
                        a4[:, :, col_off:col_off + 128],
                        t4[:, :, :])
                pending_av.append((t, col_off, attnT))
                if len(pending_av) > 3:
                    emit_av(*pending_av.pop(0))
            for args in pending_av:
                emit_av(*args)

            # ---- group epilogue: normalize + output projection ----
            av_sb = small.tile([128, 512], f32, tag="av_sb")
            nc.vector.tensor_copy(av_sb[:], av[:])
            rcp = small.tile([128, 512], f32, tag="rcp")
            nc.vector.reciprocal_approx_fast(out=rcp[:], in_=av_sb[:])
            bc = misc_ps.tile([128, 512], f32, tag="misc")
            nc.tensor.matmul(out=bc[:], lhsT=sel[:].bitcast(f32), rhs=rcp[:],
                             start=True, stop=True)
            avn = small.tile([128, 512], f32r, tag="avn")
            nc.vector.tensor_mul(avn[:], av_sb[:], bc[:])
            yp = misc_ps.tile([64, 512], f32, tag="misc")
            nc.tensor.matmul(out=yp[:], lhsT=wo[:], rhs=avn[:], start=True, stop=False)
            nc.tensor.matmul(out=yp[:], lhsT=bo2[:], rhs=ones_row[:], start=False, stop=True)
            y_sb = small.tile([64, 512], f32, tag="y_sb")
            nc.vector.tensor_copy(y_sb[:], yp[:])
            nc.sync.dma_start(out=y_d[:, 512 * g:512 * (g + 1)], in_=y_sb[:])

    nc.compile()
    return nc


def _prep_inputs(queries, keys, values, Wq, bq, Wk, bk, Wv, bv, Wo, bo):
    """Build the 8 per-core input maps (host-side layout/sharding only)."""
    ones = np.ones((1, L), np.float32)

    def aug_t(x_b):  # [L, 64] -> [65, L]
        return np.ascontiguousarray(np.vstack([x_b.T, ones]).astype(np.float32))

    # padded projection weights per quad: col 32r+d <- head (4Q+r) dim d
    def w_pad(W, b, quad, scale=1.0):
        out = np.zeros((65, 128), np.float32)
        for r in range(4):
            ch = 8 * (4 * quad + r)
            out[:64, 32 * r:32 * r + 8] = W[ch:ch + 8, :].T * scale
            out[64, 32 * r:32 * r + 8] = b[ch:ch + 8] * scale
        return out

    def wv_aug(quad):  # [65, 36]: col 9c+e <- head (4Q+c) dim e; col 9c+8 = e64
        out = np.zeros((65, 36), np.float32)
        for c in range(4):
            ch = 8 * (4 * quad + c)
            out[:64, 9 * c:9 * c + 8] = Wv[ch:ch + 8, :].T
            out[64, 9 * c:9 * c + 8] = bv[ch:ch + 8]
            out[64, 9 * c + 8] = 1.0
        return out

    def wo_pad(quad):  # [128, 64]: row 32c+d -> Wo[:, 8(4Q+c)+d]
        out = np.zeros((128, 64), np.float32)
        for c in range(4):
            ch = 8 * (4 * quad + c)
            out[32 * c:32 * c + 8, :] = Wo[:, ch:ch + 8].T
        return out

    import ml_dtypes
    tri = (np.arange(128)[:, None] <= np.arange(128)[None, :]).astype(np.float32)
    tri4 = np.ascontiguousarray(np.tile(tri, (1, 4)).astype(ml_dtypes.bfloat16))
    sel = np.zeros((128, 128), np.float32)
    for c in range(4):
        sel[32 * c + 8, 32 * c:32 * c + 9] = 1.0
    bo2 = (bo.astype(np.float32) / 2.0).reshape(1, 64)

    w_cache = {}
    for q in range(2):
        w_cache[q] = dict(
            wq=w_pad(Wq, bq, q, scale=float(SCALE)),
            wk=w_pad(Wk, bk, q),
            wv=wv_aug(q),
            wo=wo_pad(q),
        )

    in_maps = []
    for c in range(8):
        b, q = c // 2, c % 2
        in_maps.append(dict(
            xq=aug_t(np.asarray(queries[b])),
            xk=aug_t(np.asarray(keys[b])),
            xv=aug_t(np.asarray(values[b])),
            wq=w_cache[q]["wq"], wk=w_cache[q]["wk"],
            wv=w_cache[q]["wv"], wo=w_cache[q]["wo"],
            bo2=bo2, tri=tri4, sel=sel, ones=np.ones((1, 512), np.float32),
        ))
    return in_maps


def _install_trace_hook():
    import contextlib
    import ctypes
    import types

    name = "antenv.axon_hooks"
    if name in sys.modules:
        return
    so_path = "/opt/axon/libaxon_pjrt.so"
    if not os.path.exists(so_path):
        return
    lib = ctypes.CDLL(so_path)
    if not hasattr(lib, "axon_start_nrt_profile"):
        return
    lib.axon_start_nrt_profile.argtypes = [ctypes.POINTER(ctypes.c_int64), ctypes.c_size_t]
    lib.axon_start_nrt_profile.restype = ctypes.c_int64
    lib.axon_stop_nrt_profile.argtypes = [ctypes.c_char_p]
    lib.axon_stop_nrt_profile.restype = ctypes.c_int64

    @contextlib.contextmanager
    def _hook(output_dir, device_ids):
        import jax
        jax.devices()
        if device_ids:
            ids = (ctypes.c_int64 * len(device_ids))(*device_ids)
            rc = lib.axon_start_nrt_profile(ids, len(device_ids))
        else:
            rc = lib.axon_start_nrt_profile(None, 0)
        if rc != 0:
            raise RuntimeError(f"axon_start_nrt_profile rc={rc}")
        try:
            yield
        finally:
            n = lib.axon_stop_nrt_profile(str(output_dir).encode())
            print(f"profile: {n} file(s) in {output_dir}", file=sys.stderr)

    mod = types.ModuleType(name)
    mod._hook = _hook
    mod.set_axon_ntff_profile_hook = lambda h: setattr(mod, "_hook", h)
    mod.get_axon_ntff_profile_hook = lambda: mod._hook
    sys.modules[name] = mod


def kernel(queries, keys, values, attention_mask, Wq, bq, Wk, bk, Wv, bv, Wo, bo):
    global LAST_EXEC_NS
    from concourse.bass_utils import run_bass_kernel_spmd

    causal = bool(int(np.asarray(attention_mask)))
    if causal not in _CACHE:
        _CACHE[causal] = _build(causal)
    nc = _CACHE[causal]

    in_maps = _prep_inputs(queries, keys, values, Wq, bq, Wk, bk, Wv, bv, Wo, bo)

    trace = os.environ.get("KERNEL_TRACE", "") == "1"
    kwargs = {}
    if trace:
        _install_trace_hook()
        kwargs = dict(trace=True, tmpdir=os.environ.get("KERNEL_TRACE_DIR") or None)
    res = run_bass_kernel_spmd(nc, in_maps, core_ids=list(range(8)), **kwargs)
    LAST_EXEC_NS = res.exec_time_ns

    out = np.empty((B, L, D), np.float32)
    for b in range(B):
        out[b] = (res.results[2 * b]["y"] + res.results[2 * b + 1]["y"]).T
    return out
